# revision 1
# baseline (speedup 1.0000x reference)
"""Trainium2 Bass kernel for ExtractorLoss (PSD SNR loss).

loss = -mean_b( 10*log10( (mean wanted psd) / (mean unwanted psd) ) )
with psd[b,g] = (x @ cos_g)^2 + (x @ sin_g)^2 over a 201-bin frequency grid.

Math: grid frequencies are g/1800 cycles/sample (g = grid_bpm in 40..240,
fs = 30 Hz), so the DFT basis has period 1800 over t and half-period sign
symmetry: cos(2*pi*g*(tau+900j)/1800) = (-1)^{gj} cos(2*pi*g*tau/1800).
Folding the four 900-sample segments of x on host (plain sum for even g,
alternating sum for odd g) shrinks the GEMM contraction from 3600 to 900
with two parity classes — 4x less PE work and x DMA than the naive GEMM.

Sharding: data-parallel over batch across 8 NeuronCores (512 rows each).
Host packs, per core, a [900, 1428] bf16 tensor: [xeT(512) | xoT(512) |
basis_e(202) | basis_o(202)] so each contraction k-tile is one large DMA.
Odd-parity bins (100) are zero-padded to 101 so even/odd blocks align.

Raw Bacc kernel (no TileContext — avoids its ~12us semaphore-clear +
barrier tail): the Sync engine streams the 8 k-tile DMAs with a window-2
issue throttle (so the first tile completes early instead of round-robin
sharing bandwidth with the whole stream), PE accumulates both parity
GEMMs into 8 PSUM banks, then a per-row-tile interleaved epilogue runs
Square on ScalarE and fused multiply-reduce (tensor_tensor_reduce) on
DVE. The tiny log/mean runs on host in float64.
"""

import functools
import sys

import numpy as np
import ml_dtypes

if "/opt/trn_rl_repo" not in sys.path:
    sys.path.insert(0, "/opt/trn_rl_repo")

# Problem constants (fixed by the problem spec).
B, T, NG = 4096, 3600, 201
NCORES = 8
BS = B // NCORES          # 512 batch rows per core
MT = BS // 128            # 4 output partition tiles per core
TF = T // 4               # 900 folded contraction length
KT = 128
NK = (TF + KT - 1) // KT  # 8 k-tiles (7x128 + 1x4)
NGE = 101                 # even-parity bins (grid 40,42..240)
NGO = 100                 # odd-parity bins (grid 41,43..239)
NGP = 101                 # odd padded to 101
NBH = 2 * NGP             # 202 basis cols per parity (cos|sin)
XCOL = 2 * BS             # 1024 x cols (xe | xo)
PCOL = XCOL + 2 * NBH     # 1428 packed cols per k row

BF16 = ml_dtypes.bfloat16


@functools.lru_cache(maxsize=1)
def _build_program():
    import concourse.bacc as bacc
    import concourse.mybir as mybir
    from contextlib import ExitStack

    f32 = mybir.dt.float32
    bf16 = mybir.dt.bfloat16

    # Bacc (not raw Bass): its finalize() legalizes multi-wait instructions
    # into event-semaphore chains — walrus allows only 1 wait per inst.
    nc = bacc.Bacc()
    xb = nc.declare_dram_parameter("xb", [TF, PCOL], bf16, isOutput=False)
    maskd = nc.declare_dram_parameter("mask", [BS, 2 * NBH], bf16, isOutput=False)
    outd = nc.declare_dram_parameter("out", [BS, 2], f32, isOutput=True)

    ksizes = []
    off = 0
    while off < TF:
        sz = min(KT, TF - off)
        ksizes.append((off, sz))
        off += sz

    with ExitStack() as ctx:
        xsb = ctx.enter_context(nc.sbuf_tensor("xsb", [128, NK, PCOL], bf16))
        masksb = ctx.enter_context(nc.sbuf_tensor("masksb", [128, MT, 2, NBH], bf16))
        sq = ctx.enter_context(nc.sbuf_tensor("sq", [128, MT, 2, NBH], f32))
        msq = ctx.enter_context(nc.sbuf_tensor("msq", [128, MT, 2, NBH], f32))
        outsb = ctx.enter_context(nc.sbuf_tensor("outsb", [128, MT, 2], f32))
        ps = ctx.enter_context(nc.psum_tensor("ps", [128, MT, 2, 512], f32))

        dsems = [
            ctx.enter_context(nc.semaphore(f"dsem{k}")) for k in range(NK)
        ]
        msem = ctx.enter_context(nc.semaphore("msem"))
        pesem = ctx.enter_context(nc.semaphore("pesem"))
        actsem = ctx.enter_context(nc.semaphore("actsem"))
        dvesem = ctx.enter_context(nc.semaphore("dvesem"))
        dv2 = ctx.enter_context(nc.semaphore("dv2"))
        osem = ctx.enter_context(nc.semaphore("osem"))

        # Raw semaphores are not cleared on allocation; zero them before any
        # engine waits (runs in the preamble block, then a full barrier).
        allsems = dsems + [msem, pesem, actsem, dvesem, osem]
        nums = sorted(sh.num for sh in allsems)
        lo = 0
        while lo < len(nums):
            hi = lo
            while hi + 1 < len(nums) and nums[hi + 1] == nums[hi] + 1:
                hi += 1
            nc.gpsimd.sem_clear(range(nums[lo], nums[hi] + 1))
            lo = hi + 1
        nc.all_engine_barrier()

        block = ctx.enter_context(nc.Block())

        @block.sync
        def _(sync):
            for k, (off, sz) in enumerate(ksizes):
                nc.sync.dma_start(
                    out=xsb[:sz, k, :], in_=xb[off : off + sz, :]
                ).then_inc(dsems[k], 16)
            nc.sync.dma_start(
                out=masksb[:], in_=maskd.rearrange("(m p) g -> p m g", p=128)
            ).then_inc(msem, 16)
            sync.wait_ge(dvesem, 2)
            nc.sync.dma_start(
                out=outd.rearrange("(m p) c -> p m c", p=128),
                in_=outsb[:],
            ).then_inc(osem, 16)
            sync.wait_ge(osem, 16)

        @block.tensor
        def _(tensor):
            last = None
            for k, (off, sz) in enumerate(ksizes):
                tensor.wait_ge(dsems[k], 16)
                start, stop = (k == 0), (k == NK - 1)
                for m in range(MT):
                    last = nc.tensor.matmul(
                        ps[:, m, 0, 0:NBH],
                        lhsT=xsb[:sz, k, m * 128 : (m + 1) * 128],
                        rhs=xsb[:sz, k, XCOL : XCOL + NBH],
                        start=start,
                        stop=stop,
                    )
                    last = nc.tensor.matmul(
                        ps[:, m, 1, 0:NBH],
                        lhsT=xsb[:sz, k, BS + m * 128 : BS + (m + 1) * 128],
                        rhs=xsb[:sz, k, XCOL + NBH : XCOL + 2 * NBH],
                        start=start,
                        stop=stop,
                    )
            # Matmuls complete in program order; one inc on the last is sound.
            last.then_inc(pesem, 1)

        @block.scalar
        def _(scalar):
            scalar.wait_ge(pesem, 1)
            # cos^2 of all 8 banks -> sq[:, :, 0, :] (as [128, 4, 2, 101])
            nc.scalar.activation(
                sq[:, :, 0, :].rearrange("p m (two x) -> p m two x", two=2),
                ps[:, :, :, 0:NGP],
                mybir.ActivationFunctionType.Square,
            ).then_inc(actsem, 1)
            nc.scalar.activation(
                sq[:, :, 1, :].rearrange("p m (two x) -> p m two x", two=2),
                ps[:, :, :, NGP:NBH],
                mybir.ActivationFunctionType.Square,
            ).then_inc(actsem, 1)

        @block.vector
        def _(vector):
            vector.wait_ge(msem, 16)
            # masked squares per parity block (read only ACT/DMA data)
            vector.wait_ge(actsem, 1)
            nc.vector.tensor_mul(
                msq[:, :, 0], sq[:, :, 0], masksb[:, :, 0]
            )
            vector.wait_ge(actsem, 2)
            last_mul = nc.vector.tensor_mul(
                msq[:, :, 1], sq[:, :, 1], masksb[:, :, 1]
            )
            # totals read only ACT data -> safe before the drain
            nc.vector.tensor_reduce(
                outsb[:, :, 0],
                sq[:].rearrange("p m two x -> p m (two x)"),
                axis=mybir.AxisListType.X,
                op=mybir.AluOpType.add,
            ).then_inc(dvesem, 1)
            # drain DVE pipeline before reading its own msq outputs
            last_mul.then_inc(dv2, 1)
            vector.wait_ge(dv2, 1)
            nc.vector.tensor_reduce(
                outsb[:, :, 1],
                msq[:].rearrange("p m two x -> p m (two x)"),
                axis=mybir.AxisListType.X,
                op=mybir.AluOpType.add,
            ).then_inc(dvesem, 1)

    # Run Bacc's compile passes (register allocation, 1-wait legalization
    # via event-semaphore chains) — the PJRT exec path doesn't finalize.
    nc.finalize()
    return nc


def _host_prep(x, f_true_bpm, fs, delta_bpm, sampling_bpm, fmin_bpm, fmax_bpm):
    fs = int(fs)
    delta = int(delta_bpm)
    samp = int(sampling_bpm)
    fmin = int(fmin_bpm)
    fmax = int(fmax_bpm)

    n_grid = (fmax - fmin) // samp + 1
    assert n_grid == NG and fs == 30 and samp == 1, (n_grid, fs, samp)
    grid_bpm = fmin + samp * np.arange(n_grid, dtype=np.int64)
    ge = grid_bpm[grid_bpm % 2 == 0]  # 101 even bins
    go = grid_bpm[grid_bpm % 2 == 1]  # 100 odd bins

    # Folded basis over tau in [0, 900): theta = 2*pi*g*tau/1800.
    tau = np.arange(TF, dtype=np.float64)
    thE = 2.0 * np.pi * ge[:, None] * tau[None, :] / 1800.0  # [101, 900]
    thO = 2.0 * np.pi * go[:, None] * tau[None, :] / 1800.0  # [100, 900]
    basis = np.zeros((TF, 2 * NBH), dtype=BF16)
    basis[:, 0:NGE] = np.cos(thE).T.astype(BF16)
    basis[:, NGP : NGP + NGE] = np.sin(thE).T.astype(BF16)
    basis[:, NBH : NBH + NGO] = np.cos(thO).T.astype(BF16)
    basis[:, NBH + NGP : NBH + NGP + NGO] = np.sin(thO).T.astype(BF16)

    # Fold x: 4 segments of 900; even g sums plain, odd g alternates.
    s = x.astype(np.float64).reshape(B, 4, TF)
    xe = (s[:, 0] + s[:, 1] + s[:, 2] + s[:, 3]).astype(BF16)  # [B, 900]
    xo = (s[:, 0] - s[:, 1] + s[:, 2] - s[:, 3]).astype(BF16)

    # Wanted-band masks in [maskE(101) | maskO(100) pad] layout, doubled so
    # one elementwise multiply covers both (cos^2, sin^2) blocks.
    f64 = f_true_bpm.astype(np.int64)
    mask = np.zeros((B, 2, NBH), dtype=BF16)
    mask[:, 0, 0:NGE] = np.abs(ge[None, :] - f64[:, None]) <= delta
    mask[:, 0, NGP : NGP + NGO] = np.abs(go[None, :] - f64[:, None]) <= delta
    mask[:, 1, :] = mask[:, 0, :]
    mask = mask.reshape(B, 2 * NBH)

    in_maps = []
    for c in range(NCORES):
        sl = slice(c * BS, (c + 1) * BS)
        xbp = np.empty((TF, PCOL), dtype=BF16)
        xbp[:, 0:BS] = xe[sl].T
        xbp[:, BS:XCOL] = xo[sl].T
        xbp[:, XCOL:] = basis
        in_maps.append(
            {"xb": xbp, "mask": np.ascontiguousarray(mask[sl])}
        )

    n_wanted = 2 * delta // samp + 1
    n_unwanted = n_grid - n_wanted
    return in_maps, n_wanted, n_unwanted


def _finalize(outs, n_wanted, n_unwanted):
    # outs: list of [BS, 2] fp32 per core with (total_sum, wanted_sum) rows.
    full = np.concatenate(outs, axis=0).astype(np.float64)  # [B, 2]
    total, wanted = full[:, 0], full[:, 1]
    term1 = wanted / n_wanted
    term2 = (total - wanted) / n_unwanted
    snr = 10.0 * np.log10(term1 / term2)
    return np.array(-snr.mean(), dtype=np.float32)


def kernel(x, f_true_bpm, fs, delta_bpm, sampling_bpm, fmin_bpm, fmax_bpm):
    from concourse.bass_utils import run_bass_kernel_spmd

    x = np.asarray(x, dtype=np.float32)
    f_true_bpm = np.asarray(f_true_bpm)
    in_maps, n_wanted, n_unwanted = _host_prep(
        x, f_true_bpm, fs, delta_bpm, sampling_bpm, fmin_bpm, fmax_bpm
    )
    nc = _build_program()
    res = run_bass_kernel_spmd(nc, in_maps, core_ids=list(range(NCORES)))
    outs = [r["out"] for r in res.results]
    return _finalize(outs, n_wanted, n_unwanted)



# revision 7
# speedup vs baseline: 1.3593x; 1.3593x over previous
"""Trainium2 Bass kernel for ExtractorLoss (PSD SNR loss).

loss = -mean_b( 10*log10( (mean wanted psd) / (mean unwanted psd) ) )
with psd[b,g] = (x @ cos_g)^2 + (x @ sin_g)^2 over a 201-bin frequency grid.

Math: grid frequencies are g/1800 cycles/sample (g = grid_bpm in 40..240,
fs = 30 Hz), so the DFT basis has period 1800 and quarter-period structure:
shifting tau by 450 multiplies (cos,sin) by a rotation of angle (pi/2)g.
Folding the eight 450-sample segments of x on host gives, per batch row,
four folded vectors (u0 for g%4==0, u2 for g%4==2, uo/vo for odd g) and a
GEMM with contraction 450 — 9696 PE column-cycles per core vs 12928 for the
half-period fold and 51712 for the naive GEMM.

Inputs are quantized to fp8 e4m3 on host (validated: final rel err ~6e-4
vs the 2e-2 gate); this halves HBM traffic. Per core the packed x tensor is
[450, 2650] = [u0|u2|uo|vo (4x512 batch cols) | even basis 202 | odd-u 202
... odd bases 200+200], stored k-tile-major so each of 4 DMAs lands one
128-row contraction tile with one contiguous ~2.6KB descriptor/partition.

Raw Bacc kernel, no TileContext and no Block barriers: the NEFF-level
epilogue (per-semaphore resets, ~6us on the Tensor engine) starts on each
engine as soon as its own stream ends, so engines that finish early hide
the fixed epilogue under the remaining work. All cross-engine ordering is
explicit semaphores; hardware semaphores start at 0 (walrus clears all of
S[3..255] in its own epilogue every execution, and the runtime zeroes them
at NEFF load).

Pipeline: Sync streams 4 k-tile DMAs; PE runs ~20 zero matmuls first (fed
by a GpSimd memset) purely to flip the HAM clock gate from 1.2 to 2.4 GHz
before real data lands, then accumulates 4 matmuls per (k,m) into one PSUM
bank per m-tile; Scalar fetches the mask (after the x stream clears) and
runs Square-with-accumulate per m-tile (sq -> SBUF bf16, total -> f32);
DVE does one fused tensor_tensor_reduce per m-tile (sq*mask, sum) for the
wanted-band sum; Sync DMAs the [128,8] result out. Log/mean run on host.
"""

import functools
import sys

import numpy as np
import ml_dtypes

if "/opt/trn_rl_repo" not in sys.path:
    sys.path.insert(0, "/opt/trn_rl_repo")

# Problem constants (fixed by the problem spec).
B, T, NG = 4096, 3600, 201
NCORES = 8
BS = B // NCORES          # 512 batch rows per core
MT = BS // 128            # 4 output partition tiles per core
TF = T // 8               # 450 folded contraction length
N0, N2, NO = 51, 50, 100  # grid bins with g%4==0 / g%4==2 / odd g
XCOL = 4 * BS             # 2048 x cols (u0|u2|uo|vo)
BE0 = XCOL                # even-class basis [C0|S0|C2|S2]
BOU = BE0 + 2 * (N0 + N2)        # odd basis for uo [Co|So]
BOV = BOU + 2 * NO               # odd basis for vo [-sgn*So|sgn*Co]
PCOL = BOV + 2 * NO              # 2650 packed cols per fold row
FD = 2 * (N0 + N2 + NO)          # 402 psd cols per m-tile
KS = [128, 128, 128, TF - 384]   # contraction k-tiles (128,128,128,66)

E4M3 = ml_dtypes.float8_e4m3
BF16 = ml_dtypes.bfloat16
NWARM = 20                # HAM warmup matmuls (~2.1us at the cold rate)


@functools.lru_cache(maxsize=1)
def _build_program():
    import concourse.bacc as bacc
    import concourse.mybir as mybir
    from contextlib import ExitStack

    f32 = mybir.dt.float32
    bf16 = mybir.dt.bfloat16
    fp8 = mybir.dt.float8e4

    nc = bacc.Bacc()
    xq = nc.declare_dram_parameter("xq", [128, 3, PCOL], fp8, isOutput=False)
    xr = nc.declare_dram_parameter("xr", [KS[3], PCOL], fp8, isOutput=False)
    maskd = nc.declare_dram_parameter("mask", [128, MT, FD], bf16, isOutput=False)
    outd = nc.declare_dram_parameter("out", [128, 2 * MT], f32, isOutput=True)

    with ExitStack() as ctx:
        xsb = ctx.enter_context(nc.sbuf_tensor("xsb", [128, 4, PCOL], fp8))
        masksb = ctx.enter_context(nc.sbuf_tensor("masksb", [128, MT, FD], bf16))
        sq = ctx.enter_context(nc.sbuf_tensor("sq", [128, MT, FD], bf16))
        junk = ctx.enter_context(nc.sbuf_tensor("junk", [128, MT, FD], bf16))
        outsb = ctx.enter_context(nc.sbuf_tensor("outsb", [128, 2 * MT], f32))
        warm = ctx.enter_context(nc.sbuf_tensor("warm", [128, 128], bf16))
        ps = ctx.enter_context(nc.psum_tensor("ps", [128, 8, 512], f32))

        dsems = [ctx.enter_context(nc.semaphore(f"dsem{k}")) for k in range(4)]
        msem = ctx.enter_context(nc.semaphore("msem"))
        wsem = ctx.enter_context(nc.semaphore("wsem"))
        pesem = ctx.enter_context(nc.semaphore("pesem"))
        actsem = ctx.enter_context(nc.semaphore("actsem"))
        dvesem = ctx.enter_context(nc.semaphore("dvesem"))
        osem = ctx.enter_context(nc.semaphore("osem"))

        # --- GpSimd: zero the warmup operand, then done for the run.
        nc.gpsimd.memset(warm[:], 0).then_inc(wsem, 1)

        # --- Sync: stream the x k-tiles, then write the result out.
        for k in range(3):
            nc.sync.dma_start(out=xsb[:, k, :], in_=xq[:, k, :]).then_inc(
                dsems[k], 16
            )
        nc.sync.dma_start(out=xsb[: KS[3], 3, :], in_=xr[:, :]).then_inc(
            dsems[3], 16
        )
        nc.sync.wait_ge(dvesem, 1)
        nc.sync.dma_start(out=outd[:], in_=outsb[:]).then_inc(osem, 16)
        nc.sync.wait_ge(osem, 16)

        # --- Scalar: mask DMA (after the x stream is clear of the rings),
        # then per-m Square with accumulated row totals.
        nc.scalar.wait_ge(dsems[1], 16)
        nc.scalar.dma_start(out=masksb[:], in_=maskd[:]).then_inc(msem, 16)
        for m in range(MT):
            nc.scalar.wait_ge(pesem, m + 1)
            nc.scalar.activation(
                sq[:, m],
                ps[:, m, 0:FD],
                mybir.ActivationFunctionType.Square,
                accum_out=outsb[:, m : m + 1],
            ).then_inc(actsem, 1)

        # --- Tensor: HAM warmup on zeros, then the folded-DFT GEMM.
        nc.tensor.wait_ge(wsem, 1)
        for _ in range(NWARM):
            nc.tensor.matmul(
                ps[:, 4, 0:128], lhsT=warm[:], rhs=warm[:], start=True, stop=True
            )
        for k in range(4):
            kk = KS[k]
            nc.tensor.wait_ge(dsems[k], 16)
            for m in range(MT):
                c = m * 128
                # One accumulation group per PSUM bank: the first matmul
                # (start=True) clears the whole bank, the last (stop=True)
                # closes the group; everything between accumulates.
                nc.tensor.matmul(
                    ps[:, m, 0 : 2 * N0],
                    lhsT=xsb[:kk, k, c : c + 128],
                    rhs=xsb[:kk, k, BE0 : BE0 + 2 * N0],
                    start=(k == 0),
                    stop=False,
                )
                nc.tensor.matmul(
                    ps[:, m, 2 * N0 : 2 * (N0 + N2)],
                    lhsT=xsb[:kk, k, BS + c : BS + c + 128],
                    rhs=xsb[:kk, k, BE0 + 2 * N0 : BOU],
                    start=False,
                    stop=False,
                )
                nc.tensor.matmul(
                    ps[:, m, 2 * (N0 + N2) : FD],
                    lhsT=xsb[:kk, k, 2 * BS + c : 2 * BS + c + 128],
                    rhs=xsb[:kk, k, BOU:BOV],
                    start=False,
                    stop=False,
                )
                last = nc.tensor.matmul(
                    ps[:, m, 2 * (N0 + N2) : FD],
                    lhsT=xsb[:kk, k, 3 * BS + c : 3 * BS + c + 128],
                    rhs=xsb[:kk, k, BOV:PCOL],
                    start=False,
                    stop=(k == 3),
                )
                if k == 3:
                    last.then_inc(pesem, 1)

        # --- DVE: fused (sq * mask) multiply-reduce per m for the wanted sum.
        nc.vector.wait_ge(msem, 16)
        for m in range(MT):
            nc.vector.wait_ge(actsem, m + 1)
            stt = nc.vector.scalar_tensor_tensor(
                out=junk[:, m],
                in0=sq[:, m],
                scalar=1.0,
                in1=masksb[:, m],
                op0=mybir.AluOpType.mult,
                op1=mybir.AluOpType.mult,
                accum_out=outsb[:, MT + m : MT + m + 1],
            )
        stt.then_inc(dvesem, 1)

    nc.finalize()
    return nc


def _host_prep(x, f_true_bpm, fs, delta_bpm, sampling_bpm, fmin_bpm, fmax_bpm):
    fs = int(fs)
    delta = int(delta_bpm)
    samp = int(sampling_bpm)
    fmin = int(fmin_bpm)
    fmax = int(fmax_bpm)

    n_grid = (fmax - fmin) // samp + 1
    assert n_grid == NG and fs == 30 and samp == 1, (n_grid, fs, samp)
    grid = fmin + samp * np.arange(n_grid, dtype=np.int64)
    g0 = grid[grid % 4 == 0]          # 51 bins
    g2 = grid[grid % 4 == 2]          # 50 bins
    go = grid[grid % 2 == 1]          # 100 bins
    assert len(g0) == N0 and len(g2) == N2 and len(go) == NO

    # Quarter-period folded basis over tau in [0, 450).
    tau = np.arange(TF, dtype=np.float64)
    th = lambda g: 2.0 * np.pi * tau[:, None] * g[None, :] / 1800.0
    C0, S0 = np.cos(th(g0)), np.sin(th(g0))
    C2, S2 = np.cos(th(g2)), np.sin(th(g2))
    Co, So = np.cos(th(go)), np.sin(th(go))
    sgn = np.where(go % 4 == 1, 1.0, -1.0)[None, :]
    basis = np.empty((TF, PCOL - XCOL), dtype=np.float64)
    basis[:, 0:N0] = C0
    basis[:, N0 : 2 * N0] = S0
    basis[:, 2 * N0 : 2 * N0 + N2] = C2
    basis[:, 2 * N0 + N2 : 2 * (N0 + N2)] = S2
    o = 2 * (N0 + N2)
    basis[:, o : o + NO] = Co
    basis[:, o + NO : o + 2 * NO] = So
    basis[:, o + 2 * NO : o + 3 * NO] = -sgn * So
    basis[:, o + 3 * NO : o + 4 * NO] = sgn * Co
    basis8 = basis.astype(E4M3)

    # Fold x: 8 segments of 450 with per-class segment coefficients.
    s = x.astype(np.float64).reshape(B, 8, TF)
    e, oo = s[:, 0::2], s[:, 1::2]     # even/odd segment groups [B,4,TF]
    u0 = (e.sum(1) + oo.sum(1)).astype(E4M3)
    u2 = (e.sum(1) - oo.sum(1)).astype(E4M3)
    alt = np.array([1.0, -1.0, 1.0, -1.0])
    uo = np.einsum("j,bjt->bt", alt, e).astype(E4M3)
    vo = np.einsum("j,bjt->bt", alt, oo).astype(E4M3)

    # Wanted-band mask in [E0|E0|E2|E2|O|O] column order, bf16.
    f64 = f_true_bpm.astype(np.int64)
    w0 = np.abs(g0[None, :] - f64[:, None]) <= delta
    w2 = np.abs(g2[None, :] - f64[:, None]) <= delta
    wo = np.abs(go[None, :] - f64[:, None]) <= delta
    mask = np.concatenate([w0, w0, w2, w2, wo, wo], axis=1).astype(BF16)

    in_maps = []
    for c in range(NCORES):
        sl = slice(c * BS, (c + 1) * BS)
        xbp = np.empty((TF, PCOL), dtype=E4M3)
        xbp[:, 0:BS] = u0[sl].T
        xbp[:, BS : 2 * BS] = u2[sl].T
        xbp[:, 2 * BS : 3 * BS] = uo[sl].T
        xbp[:, 3 * BS : XCOL] = vo[sl].T
        xbp[:, XCOL:] = basis8
        # k-tile-major: partition p holds fold rows p, 128+p, 256+p.
        xqc = np.ascontiguousarray(
            xbp[0:384].reshape(3, 128, PCOL).transpose(1, 0, 2)
        )
        xrc = np.ascontiguousarray(xbp[384:TF])
        mc = np.ascontiguousarray(
            mask[sl].reshape(MT, 128, FD).transpose(1, 0, 2)
        )
        in_maps.append({"xq": xqc, "xr": xrc, "mask": mc})

    n_wanted = 2 * delta // samp + 1
    n_unwanted = n_grid - n_wanted
    return in_maps, n_wanted, n_unwanted


def _finalize(outs, n_wanted, n_unwanted):
    # outs: per core [128, 8] f32 = [total m0..m3 | wanted m0..m3] per row.
    snrs = []
    for o in outs:
        o = np.asarray(o, dtype=np.float64)
        total = o[:, 0:MT].T.reshape(-1)    # batch row m*128+p
        wanted = o[:, MT : 2 * MT].T.reshape(-1)
        term1 = wanted / n_wanted
        term2 = (total - wanted) / n_unwanted
        snrs.append(10.0 * np.log10(term1 / term2))
    return np.array(-np.concatenate(snrs).mean(), dtype=np.float32)


def kernel(x, f_true_bpm, fs, delta_bpm, sampling_bpm, fmin_bpm, fmax_bpm):
    from concourse.bass_utils import run_bass_kernel_spmd

    x = np.asarray(x, dtype=np.float32)
    f_true_bpm = np.asarray(f_true_bpm)
    in_maps, n_wanted, n_unwanted = _host_prep(
        x, f_true_bpm, fs, delta_bpm, sampling_bpm, fmin_bpm, fmax_bpm
    )
    nc = _build_program()
    res = run_bass_kernel_spmd(nc, in_maps, core_ids=list(range(NCORES)))
    outs = [r["out"] for r in res.results]
    return _finalize(outs, n_wanted, n_unwanted)


# revision 13
# speedup vs baseline: 1.5343x; 1.1287x over previous
"""Trainium2 Bass kernel for ExtractorLoss (PSD SNR loss).

loss = -mean_b( 10*log10( (mean wanted psd) / (mean unwanted psd) ) )
with psd[b,g] = (x @ cos_g)^2 + (x @ sin_g)^2 over a 201-bin frequency grid.

Math: grid frequencies are g/1800 cycles/sample (g = grid_bpm in 40..240,
fs = 30 Hz), so the DFT basis has period 1800 and quarter-period structure:
shifting tau by 450 multiplies (cos,sin) by a rotation of angle (pi/2)g.
Folding the eight 450-sample segments of x on host gives, per batch row,
four folded vectors (u0 for g%4==0, u2 for g%4==2, uo/vo for odd g) and a
GEMM with contraction 450 — 9696 PE column-cycles per core vs 12928 for the
half-period fold and 51712 for the naive GEMM.

Inputs are quantized to fp8 e4m3 on host (validated: final rel err ~6e-4
vs the 2e-2 gate); this halves HBM traffic. Per core the packed x tensor is
[450, 2650] = [u0|u2|uo|vo (4x512 batch cols) | even basis 202 | odd-u 202
... odd bases 200+200], stored k-tile-major so each of 4 DMAs lands one
128-row contraction tile with one contiguous ~2.6KB descriptor/partition.

Raw Bacc kernel, no TileContext and no Block barriers: the NEFF-level
epilogue (per-semaphore resets, ~6us on the Tensor engine) starts on each
engine as soon as its own stream ends, so engines that finish early hide
the fixed epilogue under the remaining work. All cross-engine ordering is
explicit semaphores; hardware semaphores start at 0 (walrus clears all of
S[3..255] in its own epilogue every execution, and the runtime zeroes them
at NEFF load).

Pipeline: Sync streams 4 k-tile DMAs; PE runs ~20 zero matmuls first (fed
by a GpSimd memset) purely to flip the HAM clock gate from 1.2 to 2.4 GHz
before real data lands, then accumulates 4 matmuls per (k,m) into one PSUM
bank per m-tile; Scalar fetches the mask (after the x stream clears) and
runs Square-with-accumulate per m-tile (sq -> SBUF bf16, total -> f32);
DVE does one fused tensor_tensor_reduce per m-tile (sq*mask, sum) for the
wanted-band sum; Sync DMAs the [128,8] result out. Log/mean run on host.
"""

import functools
import sys

import numpy as np
import ml_dtypes

if "/opt/trn_rl_repo" not in sys.path:
    sys.path.insert(0, "/opt/trn_rl_repo")

# Problem constants (fixed by the problem spec).
B, T, NG = 4096, 3600, 201
NCORES = 8
BS = B // NCORES          # 512 batch rows per core
MT = BS // 128            # 4 output partition tiles per core
TF = T // 8               # 450 folded contraction length
N0, N2, NO = 51, 50, 100  # grid bins with g%4==0 / g%4==2 / odd g
XCOL = 4 * BS             # 2048 x cols (u0|u2|uo|vo)
BE0 = XCOL                # even-class basis [C0|S0|C2|S2]
BOU = BE0 + 2 * (N0 + N2)        # odd basis for uo [Co|So]
BOV = BOU + 2 * NO               # odd basis for vo [-sgn*So|sgn*Co]
PCOL = BOV + 2 * NO              # 2650 packed cols per fold row
FD = 2 * (N0 + N2 + NO)          # 402 psd cols per m-tile
FDP = FD + 2                     # padded stride (4B-aligns each m slice)
KS = [128, 128, 128, TF - 384]   # contraction k-tiles (128,128,128,66)

E4M3 = ml_dtypes.float8_e4m3
BF16 = ml_dtypes.bfloat16
NWARM = 20                # big HAM warmup matmuls (~2.1us at the cold rate)
NWTAIL = 16               # small tail warmups bridging to the first k-tile


@functools.lru_cache(maxsize=1)
def _build_program():
    import concourse.bacc as bacc
    import concourse.mybir as mybir
    from contextlib import ExitStack

    f32 = mybir.dt.float32
    bf16 = mybir.dt.bfloat16
    fp8 = mybir.dt.float8e4

    nc = bacc.Bacc()
    xq = nc.declare_dram_parameter("xq", [128, 3, PCOL], fp8, isOutput=False)
    xr = nc.declare_dram_parameter("xr", [KS[3], PCOL], fp8, isOutput=False)
    maskd = nc.declare_dram_parameter("mask", [128, MT, FDP], bf16, isOutput=False)
    outd = nc.declare_dram_parameter("out", [128, 2 * MT], f32, isOutput=True)

    with ExitStack() as ctx:
        xsb = ctx.enter_context(nc.sbuf_tensor("xsb", [128, 4, PCOL], fp8))
        masksb = ctx.enter_context(nc.sbuf_tensor("masksb", [128, MT, FDP], bf16))
        sq = ctx.enter_context(nc.sbuf_tensor("sq", [128, MT, FDP], bf16))
        junk = ctx.enter_context(nc.sbuf_tensor("junk", [128, MT, FDP], bf16))
        outsb = ctx.enter_context(nc.sbuf_tensor("outsb", [128, 2 * MT], f32))
        warm = ctx.enter_context(nc.sbuf_tensor("warm", [128, 128], bf16))
        ps = ctx.enter_context(nc.psum_tensor("ps", [128, 8, 512], f32))

        dsems = [ctx.enter_context(nc.semaphore(f"dsem{k}")) for k in range(4)]
        msem = ctx.enter_context(nc.semaphore("msem"))
        wsem = ctx.enter_context(nc.semaphore("wsem"))
        pesem = ctx.enter_context(nc.semaphore("pesem"))
        actsem = ctx.enter_context(nc.semaphore("actsem"))
        dvesem = ctx.enter_context(nc.semaphore("dvesem"))
        osem = ctx.enter_context(nc.semaphore("osem"))

        # --- GpSimd: zero the warmup operand, then done for the run.
        nc.gpsimd.memset(warm[:], 0).then_inc(wsem, 1)

        # --- Sync: stream the x k-tiles, then write the result out.
        for k in range(3):
            nc.sync.dma_start(out=xsb[:, k, :], in_=xq[:, k, :]).then_inc(
                dsems[k], 16
            )
        nc.sync.dma_start(out=xsb[: KS[3], 3, :], in_=xr[:, :]).then_inc(
            dsems[3], 16
        )
        # Mask rides the same HWDGE ring: strictly after the x stream, so it
        # cannot steal SDMA packet slots from the k-tiles PE is waiting on.
        nc.sync.dma_start(out=masksb[:], in_=maskd[:]).then_inc(msem, 16)
        nc.sync.wait_ge(dvesem, 1)
        nc.sync.dma_start(out=outd[:], in_=outsb[:]).then_inc(osem, 16)

        # --- Scalar: mask DMA (after the x stream is clear of the rings),
        # then per-m Square with accumulated row totals.
        for m in range(MT):
            nc.scalar.wait_ge(pesem, m + 1)
            nc.scalar.activation(
                sq[:, m, 0:FD],
                ps[:, m, 0:FD],
                mybir.ActivationFunctionType.Square,
                accum_out=outsb[:, m : m + 1],
            ).then_inc(actsem, 1)

        # --- Tensor: HAM warmup on zeros, then the folded-DFT GEMM.
        nc.tensor.wait_ge(wsem, 1)
        for _ in range(NWARM):
            nc.tensor.matmul(
                ps[:, 4, 0:128], lhsT=warm[:], rhs=warm[:], start=True, stop=True
            )
        for _ in range(NWTAIL):
            nc.tensor.matmul(
                ps[:, 4, 0:16], lhsT=warm[:], rhs=warm[:, 0:16], start=True, stop=True
            )
        for k in range(2):
            kk = KS[k]
            nc.tensor.wait_ge(dsems[k], 16)
            for m in range(MT):
                c = m * 128
                # One accumulation group per PSUM bank: the first matmul
                # (start=True) clears the whole bank, the last (stop=True)
                # closes the group; everything between accumulates.
                nc.tensor.matmul(
                    ps[:, m, 0 : 2 * N0],
                    lhsT=xsb[:kk, k, c : c + 128],
                    rhs=xsb[:kk, k, BE0 : BE0 + 2 * N0],
                    start=(k == 0),
                    stop=False,
                )
                nc.tensor.matmul(
                    ps[:, m, 2 * N0 : 2 * (N0 + N2)],
                    lhsT=xsb[:kk, k, BS + c : BS + c + 128],
                    rhs=xsb[:kk, k, BE0 + 2 * N0 : BOU],
                    start=False,
                    stop=False,
                )
                nc.tensor.matmul(
                    ps[:, m, 2 * (N0 + N2) : FD],
                    lhsT=xsb[:kk, k, 2 * BS + c : 2 * BS + c + 128],
                    rhs=xsb[:kk, k, BOU:BOV],
                    start=False,
                    stop=False,
                )
                last = nc.tensor.matmul(
                    ps[:, m, 2 * (N0 + N2) : FD],
                    lhsT=xsb[:kk, k, 3 * BS + c : 3 * BS + c + 128],
                    rhs=xsb[:kk, k, BOV:PCOL],
                    start=False,
                    stop=(k == 3),
                )
        # Last two k-tiles per m-tile back to back so each m's PSUM bank
        # closes (and its epilogue starts) as early as possible.
        nc.tensor.wait_ge(dsems[2], 16)
        nc.tensor.wait_ge(dsems[3], 16)
        for m in range(MT):
            c = m * 128
            for k in (2, 3):
                kk = KS[k]
                nc.tensor.matmul(
                    ps[:, m, 0 : 2 * N0],
                    lhsT=xsb[:kk, k, c : c + 128],
                    rhs=xsb[:kk, k, BE0 : BE0 + 2 * N0],
                    start=False,
                    stop=False,
                )
                nc.tensor.matmul(
                    ps[:, m, 2 * N0 : 2 * (N0 + N2)],
                    lhsT=xsb[:kk, k, BS + c : BS + c + 128],
                    rhs=xsb[:kk, k, BE0 + 2 * N0 : BOU],
                    start=False,
                    stop=False,
                )
                nc.tensor.matmul(
                    ps[:, m, 2 * (N0 + N2) : FD],
                    lhsT=xsb[:kk, k, 2 * BS + c : 2 * BS + c + 128],
                    rhs=xsb[:kk, k, BOU:BOV],
                    start=False,
                    stop=False,
                )
                last = nc.tensor.matmul(
                    ps[:, m, 2 * (N0 + N2) : FD],
                    lhsT=xsb[:kk, k, 3 * BS + c : 3 * BS + c + 128],
                    rhs=xsb[:kk, k, BOV:PCOL],
                    start=False,
                    stop=(k == 3),
                )
            last.then_inc(pesem, 1)

        # --- DVE: fused (sq * mask) multiply-reduce per m for the wanted sum.
        nc.vector.wait_ge(msem, 16)
        for m in range(MT):
            nc.vector.wait_ge(actsem, m + 1)
            stt = nc.vector.scalar_tensor_tensor(
                out=junk[:, m, 0:FD],
                in0=sq[:, m, 0:FD],
                scalar=1.0,
                in1=masksb[:, m, 0:FD],
                op0=mybir.AluOpType.mult,
                op1=mybir.AluOpType.mult,
                accum_out=outsb[:, MT + m : MT + m + 1],
            )
        stt.then_inc(dvesem, 1)

    nc.finalize()
    return nc


def _host_prep(x, f_true_bpm, fs, delta_bpm, sampling_bpm, fmin_bpm, fmax_bpm):
    fs = int(fs)
    delta = int(delta_bpm)
    samp = int(sampling_bpm)
    fmin = int(fmin_bpm)
    fmax = int(fmax_bpm)

    n_grid = (fmax - fmin) // samp + 1
    assert n_grid == NG and fs == 30 and samp == 1, (n_grid, fs, samp)
    grid = fmin + samp * np.arange(n_grid, dtype=np.int64)
    g0 = grid[grid % 4 == 0]          # 51 bins
    g2 = grid[grid % 4 == 2]          # 50 bins
    go = grid[grid % 2 == 1]          # 100 bins
    assert len(g0) == N0 and len(g2) == N2 and len(go) == NO

    # Quarter-period folded basis over tau in [0, 450).
    tau = np.arange(TF, dtype=np.float64)
    th = lambda g: 2.0 * np.pi * tau[:, None] * g[None, :] / 1800.0
    C0, S0 = np.cos(th(g0)), np.sin(th(g0))
    C2, S2 = np.cos(th(g2)), np.sin(th(g2))
    Co, So = np.cos(th(go)), np.sin(th(go))
    sgn = np.where(go % 4 == 1, 1.0, -1.0)[None, :]
    basis = np.empty((TF, PCOL - XCOL), dtype=np.float64)
    basis[:, 0:N0] = C0
    basis[:, N0 : 2 * N0] = S0
    basis[:, 2 * N0 : 2 * N0 + N2] = C2
    basis[:, 2 * N0 + N2 : 2 * (N0 + N2)] = S2
    o = 2 * (N0 + N2)
    basis[:, o : o + NO] = Co
    basis[:, o + NO : o + 2 * NO] = So
    basis[:, o + 2 * NO : o + 3 * NO] = -sgn * So
    basis[:, o + 3 * NO : o + 4 * NO] = sgn * Co
    basis8 = basis.astype(E4M3)

    # Fold x: 8 segments of 450 with per-class segment coefficients.
    s = x.astype(np.float64).reshape(B, 8, TF)
    e, oo = s[:, 0::2], s[:, 1::2]     # even/odd segment groups [B,4,TF]
    u0 = (e.sum(1) + oo.sum(1)).astype(E4M3)
    u2 = (e.sum(1) - oo.sum(1)).astype(E4M3)
    alt = np.array([1.0, -1.0, 1.0, -1.0])
    uo = np.einsum("j,bjt->bt", alt, e).astype(E4M3)
    vo = np.einsum("j,bjt->bt", alt, oo).astype(E4M3)

    # Wanted-band mask in [E0|E0|E2|E2|O|O] column order, bf16.
    f64 = f_true_bpm.astype(np.int64)
    w0 = np.abs(g0[None, :] - f64[:, None]) <= delta
    w2 = np.abs(g2[None, :] - f64[:, None]) <= delta
    wo = np.abs(go[None, :] - f64[:, None]) <= delta
    pad = np.zeros((B, FDP - FD))
    mask = np.concatenate([w0, w0, w2, w2, wo, wo, pad], axis=1).astype(BF16)

    in_maps = []
    for c in range(NCORES):
        sl = slice(c * BS, (c + 1) * BS)
        xbp = np.empty((TF, PCOL), dtype=E4M3)
        xbp[:, 0:BS] = u0[sl].T
        xbp[:, BS : 2 * BS] = u2[sl].T
        xbp[:, 2 * BS : 3 * BS] = uo[sl].T
        xbp[:, 3 * BS : XCOL] = vo[sl].T
        xbp[:, XCOL:] = basis8
        # k-tile-major: partition p holds fold rows p, 128+p, 256+p.
        xqc = np.ascontiguousarray(
            xbp[0:384].reshape(3, 128, PCOL).transpose(1, 0, 2)
        )
        xrc = np.ascontiguousarray(xbp[384:TF])
        mc = np.ascontiguousarray(
            mask[sl].reshape(MT, 128, FDP).transpose(1, 0, 2)
        )
        in_maps.append({"xq": xqc, "xr": xrc, "mask": mc})

    n_wanted = 2 * delta // samp + 1
    n_unwanted = n_grid - n_wanted
    return in_maps, n_wanted, n_unwanted


def _finalize(outs, n_wanted, n_unwanted):
    # outs: per core [128, 8] f32 = [total m0..m3 | wanted m0..m3] per row.
    snrs = []
    for o in outs:
        o = np.asarray(o, dtype=np.float64)
        total = o[:, 0:MT].T.reshape(-1)    # batch row m*128+p
        wanted = o[:, MT : 2 * MT].T.reshape(-1)
        term1 = wanted / n_wanted
        term2 = (total - wanted) / n_unwanted
        snrs.append(10.0 * np.log10(term1 / term2))
    return np.array(-np.concatenate(snrs).mean(), dtype=np.float32)


def kernel(x, f_true_bpm, fs, delta_bpm, sampling_bpm, fmin_bpm, fmax_bpm):
    from concourse.bass_utils import run_bass_kernel_spmd

    x = np.asarray(x, dtype=np.float32)
    f_true_bpm = np.asarray(f_true_bpm)
    in_maps, n_wanted, n_unwanted = _host_prep(
        x, f_true_bpm, fs, delta_bpm, sampling_bpm, fmin_bpm, fmax_bpm
    )
    nc = _build_program()
    res = run_bass_kernel_spmd(nc, in_maps, core_ids=list(range(NCORES)))
    outs = [r["out"] for r in res.results]
    return _finalize(outs, n_wanted, n_unwanted)
